# revision 4
# baseline (speedup 1.0000x reference)
"""MoE routing kernel for Trainium2 (8 NeuronCores, expert-parallel).

Problem: y[n] = x[n] @ W[index[n]].T + b[index[n]]
  x [16384, 1024] f32, index [16384] i32, W [8, 512, 1024] f32, b [8, 512] f32

Strategy (expert-parallel, dispatch on index during sharding):
  Core e owns expert e. The host groups rows by expert (the all-to-all
  dispatch) and packs a k-major transposed activation block; each core runs
  a dense [R,1024] @ [1024,512] matmul with its expert's weights. Results
  come back output-major ([512, R] per core) and are scattered back to the
  original row order on the host.

Device schedule per core (one NEFF, SPMD on cores 0-7):
  SBUF: w_sb [128, 8*512]   (k%128, k-tile, o)     -- stationary operands
        x_sb [128, 8*R_pad] (k%128, k-tile, row)   -- moving operands
        y_sb [128, 4*R_pad] (o%128, o-block, row)  -- output staging
  For ob in 0..3, kt in 0..7: load W block [128,128] into the PE once,
  then stream all row-chunks (<=512 rows each) through it, accumulating
  into one PSUM bank per (ob, chunk). fp16 inputs, fp32 PSUM.

  The Tile legalizer emits one LDWEIGHTS per matmul even when consecutive
  matmuls share the stationary operand; a post-schedule pass strips the
  redundant ones (verified on hardware: the PE keeps the loaded weights
  across non-self-loading matmuls), which removes the ~46ns weight-load
  bubble between back-to-back matmuls.

  DMA count is kept low (~17 vs ~59 in the row-major variant) because the
  framework epilogue serializes ~1 semaphore-reset op per DMA per engine
  (~138ns each on the PE queue) -- a 59-DMA program pays ~8us of teardown.
"""

from contextlib import ExitStack

import numpy as np

import concourse.bass as bass
import concourse.mybir as mybir
import concourse.tile as tile
from concourse import bacc
from concourse.bass_utils import run_bass_kernel_spmd

N_CORES = 8
D_IN = 1024
D_OUT = 512
KT = D_IN // 128  # 8 k-tiles
OB = D_OUT // 128  # 4 output blocks
CHUNK = 512  # moving-operand rows per matmul (PSUM bank = 512 f32/partition)

# matmul input dtypes. float16 runs the PE at 1 column/cycle with fast
# weight load and halves the input DMA; fp32 accumulation in PSUM keeps the
# error ~3e-4 relative (values are well within fp16 range).
X_DT = mybir.dt.float16
W_DT = mybir.dt.float16
Y_DT = mybir.dt.float16

# Number of PE-warmup dummy matmuls (0 disables). They run in the window
# between the engine-body start (~14.5us) and the first real matmul
# (gated by W/x DMA arrivals ~15.9us), pre-burning the HAM clock-gate's
# busy threshold so the real stream runs at 2.4 GHz sooner.
WARMUP_MMS = 3

# Strip redundant LDWEIGHTS after tile scheduling (see module docstring).
STRIP_LDW = True


def _strip_redundant_ldweights(nc):
    """Remove InstLdweights whose stationary AP matches the previous
    ldweights on the PE queue with no intervening PE-state clobber.

    Runs after TileContext exit (schedule + legalize done, semaphores
    assigned) and before nc.compile(). Only ldweights with no semaphore
    ops of their own (sync_info None) are candidates; dangling dependency
    references are remapped onto the kept ldweights.
    """
    removed = {}
    for b in nc.main_func.blocks:
        insts = list(b.instructions)
        keep = []
        prev_key = None
        prev_name = None
        for i in insts:
            tn = type(i).__name__
            if tn == "InstLdweights":
                key = str(i.ins[0])
                if (
                    prev_key is not None
                    and key == prev_key
                    and i.sync_info is None
                ):
                    removed[i.name] = prev_name
                    continue
                prev_key, prev_name = key, i.name
            elif tn == "InstMatmult":
                if getattr(i, "is_transpose", False):
                    prev_key = prev_name = None  # transpose loads the PE
            # other engines' instructions don't touch the PE weight array
            keep.append(i)
        if len(keep) != len(insts):
            b.instructions = keep
    if not removed:
        return 0
    for b in nc.main_func.blocks:
        for i in b.instructions:
            try:
                desc = list(i.descendants)
            except Exception:
                continue
            hit = {n: removed[n] for n in desc if n in removed}
            if hit:
                i.remap_dependency_names(hit)
    for name in removed:
        nc.inst_map.pop(name, None)
    return len(removed)


def _chunks(r_pad):
    out = []
    r = 0
    while r < r_pad:
        out.append((r, min(r + CHUNK, r_pad)))
        r += CHUNK
    return out


def build_nc(rt: int, x_dt=None, w_dt=None):
    """Build + compile the per-core Bass program for r_pad = rt*128 rows."""
    x_dt = x_dt or X_DT
    w_dt = w_dt or W_DT
    r_pad = rt * 128
    chunks = _chunks(r_pad)
    nc = bacc.Bacc(
        "TRN2",
        target_bir_lowering=False,
        debug=False,
        enable_asserts=False,
        num_devices=N_CORES,
    )
    f32 = mybir.dt.float32
    xT = nc.dram_tensor("xT", [128, KT * r_pad], x_dt, kind="ExternalInput").ap()
    wT = nc.dram_tensor("wT", [128, KT * D_OUT], w_dt, kind="ExternalInput").ap()
    yT = nc.dram_tensor("yT", [OB, 128, r_pad], Y_DT, kind="ExternalOutput").ap()

    with tile.TileContext(nc) as tc, ExitStack() as ctx:
        w_pool = ctx.enter_context(tc.tile_pool(name="w", bufs=1))
        x_pool = ctx.enter_context(tc.tile_pool(name="x", bufs=1))
        y_pool = ctx.enter_context(tc.tile_pool(name="y", bufs=1))
        p_pool = ctx.enter_context(tc.tile_pool(name="p", bufs=7, space="PSUM"))

        w_sb = w_pool.tile([128, KT * D_OUT], w_dt, tag="w", name="w_sb")
        x_sb = x_pool.tile([128, KT * r_pad], x_dt, tag="x", name="x_sb")
        y_sb = y_pool.tile([128, OB * r_pad], Y_DT, tag="y", name="y_sb")

        # --- input DMAs -------------------------------------------------
        # W on the scalar queue: kt 0-1 first (gates the first matmuls),
        # then 2-3, then 4-7.
        for a, b_ in ((0, 2), (2, 4), (4, KT)):
            nc.scalar.dma_start(
                w_sb[:, a * D_OUT : b_ * D_OUT], wT[:, a * D_OUT : b_ * D_OUT]
            )

        # x kt-major on three queues; kt0 split so the first chunk lands
        # early. Consumption order is kt0..kt7 (all rows each).
        def xdma(eng, kt, r0, r1):
            a = kt * r_pad + r0
            b_ = kt * r_pad + r1
            eng.dma_start(x_sb[:, a:b_], xT[:, a:b_])

        split = min(2 * CHUNK, r_pad)
        xdma(nc.sync, 0, 0, split)
        if split < r_pad:
            xdma(nc.sync, 0, split, r_pad)
        # queue plan (DMA-capable queues: sync/SP, scalar/Activation, gpsimd):
        # sync: kt0 (split), kt2, kt6; gpsimd: kt1, kt4, kt7; scalar: W, kt3, kt5
        for kt, eng in ((1, nc.gpsimd), (2, nc.sync), (3, nc.scalar),
                        (4, nc.gpsimd), (5, nc.scalar), (6, nc.sync),
                        (7, nc.gpsimd)):
            xdma(eng, kt, 0, r_pad)

        # --- PE warmup --------------------------------------------------
        if WARMUP_MMS:
            warm_pool = ctx.enter_context(tc.tile_pool(name="warm", bufs=1))
            warm_sb = warm_pool.tile([128, D_OUT], x_dt, tag="warm", name="warm_sb")
            nc.vector.memset(warm_sb[:], 0.0)
            warm_ps = p_pool.tile(
                [128, D_OUT], f32, tag="warm_ps", name="warm_ps", bufs=1
            )
            for _ in range(WARMUP_MMS):
                nc.tensor.matmul(
                    warm_ps[:], warm_sb[:, :128], warm_sb[:], start=True, stop=True
                )

        # --- main passes ------------------------------------------------
        for ob in range(OB):
            psums = [
                p_pool.tile([128, CHUNK], f32, tag="ps", name=f"ps{ob}_{c}")
                for c in range(len(chunks))
            ]
            for kt in range(KT):
                lhsT = w_sb[:, kt * D_OUT + ob * 128 : kt * D_OUT + (ob + 1) * 128]
                for c, (r0, r1) in enumerate(chunks):
                    nc.tensor.matmul(
                        psums[c][:, : r1 - r0],
                        lhsT,
                        x_sb[:, kt * r_pad + r0 : kt * r_pad + r1],
                        start=(kt == 0),
                        stop=(kt == KT - 1),
                    )
            for c, (r0, r1) in enumerate(chunks):
                nc.vector.tensor_copy(
                    y_sb[:, ob * r_pad + r0 : ob * r_pad + r1],
                    psums[c][:, : r1 - r0],
                )
            # ship this output block; split the last one so the final
            # transfer (the kernel tail) is small and on an idle queue.
            if ob < OB - 1:
                nc.scalar.dma_start(
                    yT[ob], y_sb[:, ob * r_pad : (ob + 1) * r_pad]
                )
            else:
                last0 = chunks[-1][0]
                nc.scalar.dma_start(
                    yT[ob][:, :last0],
                    y_sb[:, ob * r_pad : ob * r_pad + last0],
                )
                nc.sync.dma_start(
                    yT[ob][:, last0:],
                    y_sb[:, ob * r_pad + last0 : (ob + 1) * r_pad],
                )

    if STRIP_LDW:
        _strip_redundant_ldweights(nc)
    nc.compile()
    return nc


def make_in_maps(x, index, W, x_dt=None, w_dt=None):
    """Group rows by expert, pack per-core k-major transposed tiles.

    Returns (in_maps, rows_per_expert, rt) where rows_per_expert[e] is the
    original row indices handled by core e and rt*128 is the padded row
    count per core.
    """
    x_np = mybir.dt.np(x_dt or X_DT)
    w_np = mybir.dt.np(w_dt or W_DT)
    x = np.ascontiguousarray(x, dtype=np.float32)
    W = np.ascontiguousarray(W, dtype=np.float32)
    rows_per_expert = [np.nonzero(index == e)[0] for e in range(N_CORES)]
    max_rows = max(len(r) for r in rows_per_expert)
    rt = max((max_rows + 127) // 128, 1)
    r_pad = rt * 128

    in_maps = []
    for e in range(N_CORES):
        rows = rows_per_expert[e]
        xp = np.zeros((r_pad, D_IN), np.float32)
        xp[: len(rows)] = x[rows]
        # [R, D_IN] -> [128k, KT, R] so a partition line (fixed k%128) is
        # KT*R elements contiguous, kt-major.
        xT = np.ascontiguousarray(
            xp.reshape(r_pad, KT, 128).transpose(2, 1, 0).reshape(128, KT * r_pad),
            dtype=x_np,
        )
        # W[e] [D_OUT, D_IN] -> [128k, KT, D_OUT] -> [128, KT*D_OUT]
        wT = np.ascontiguousarray(
            W[e].T.reshape(KT, 128, D_OUT).transpose(1, 0, 2).reshape(128, -1),
            dtype=w_np,
        )
        in_maps.append({"xT": xT, "wT": wT})
    return in_maps, rows_per_expert, rt


def assemble_output(results, rows_per_expert, n_rows, index=None, b=None):
    y = np.zeros((n_rows, D_OUT), np.float32)
    for e, rows in enumerate(rows_per_expert):
        yT = results[e]["yT"].reshape(D_OUT, -1)  # [512, r_pad]
        y[rows] = yT[:, : len(rows)].T.astype(np.float32)
    if b is not None and np.any(b):
        y += np.asarray(b, np.float32)[np.asarray(index)]
    return y


def kernel(x, index, W, b):
    x = np.asarray(x)
    index = np.asarray(index, np.int32)
    W = np.asarray(W)
    b = np.asarray(b)
    in_maps, rows_per_expert, rt = make_in_maps(x, index, W)
    nc = build_nc(rt)
    res = run_bass_kernel_spmd(nc, in_maps, core_ids=list(range(N_CORES)))
    return assemble_output(res.results, rows_per_expert, x.shape[0], index, b)


# revision 9
# speedup vs baseline: 1.1865x; 1.1865x over previous
"""MoE routing kernel for Trainium2 (8 NeuronCores, expert-parallel).

Problem: y[n] = x[n] @ W[index[n]].T + b[index[n]]
  x [16384, 1024] f32, index [16384] i32, W [8, 512, 1024] f32, b [8, 512] f32

Strategy (expert-parallel, dispatch on index during sharding):
  Core e owns expert e. The host groups rows by expert (the all-to-all
  dispatch) and packs a k-major transposed activation block; each core runs
  a dense [R,1024] @ [1024,512] matmul with its expert's weights. Results
  come back output-major ([512, R] per core) and are scattered back to the
  original row order on the host.

Device schedule per core (one NEFF, SPMD on cores 0-7):
  SBUF: w_sb [128, 8*512]   (k%128, k-tile, o)     -- stationary operands
        x_sb [128, 8*R_pad] (k%128, k-tile, row)   -- moving operands
        y_sb [128, 4*R_pad] (o%128, o-block, row)  -- output staging

  W-stationary matmuls: the PE loads a [128,128] W block and streams row
  chunks through it. Work is organized in "waves" of 8 PSUM groups =
  {4 output blocks} x {2 row chunks of <=512}; within a wave, kt is the
  outer loop, so each x row-chunk block (128KB) is consumed by 4 matmuls
  back-to-back: instantaneous input demand ~150 GB/s, safely under the
  per-core HBM rate (~300-360 GB/s) -- no mid-stream DMA stalls.

  The Tile legalizer emits one LDWEIGHTS per matmul even when consecutive
  matmuls share the stationary operand; a post-schedule pass strips the
  redundant ones (verified on hardware: the PE keeps its loaded weights
  across non-self-loading matmuls). Back-to-back matmuls then stream at
  the pure column rate (~216ns per 512 columns at 2.4 GHz) instead of
  ~259ns with a 1:1 LDWEIGHTS:MATMUL stream.
"""

from contextlib import ExitStack

import numpy as np

import concourse.bass as bass
import concourse.mybir as mybir
import concourse.tile as tile
from concourse import bacc
from concourse.bass_utils import run_bass_kernel_spmd

N_CORES = 8
D_IN = 1024
D_OUT = 512
KT = D_IN // 128  # 8 k-tiles
OB = D_OUT // 128  # 4 output blocks
CHUNK = 512  # rows per PSUM group (PSUM bank = 512 f32 per partition)

X_DT = mybir.dt.float16
W_DT = mybir.dt.float16
Y_DT = mybir.dt.float16

# PE-warmup dummy matmuls run in the window between the engine-body start
# and the first real matmul (gated by the first W/x DMA arrivals),
# pre-burning the HAM clock-gate's busy threshold.
WARMUP_MMS = 3

# Strip redundant LDWEIGHTS after tile scheduling (see module docstring).
STRIP_LDW = True

# Skip the construction-time all-engine barrier (earlier first DMA). The
# barrier's only job is ordering the const-pool memsets (which this kernel
# never reads) before the body; Tile manages all body dependencies.
SKIP_INIT_BARRIER = False


class _NoInitBarrierBacc(bacc.Bacc):
    def all_engine_barrier(self, *, sem_only: bool = False):
        if not getattr(self, "_init_barrier_skipped", False):
            self._init_barrier_skipped = True
            return None
        return super().all_engine_barrier(sem_only=sem_only)


def _strip_redundant_ldweights(nc):
    """Remove InstLdweights whose stationary AP matches the previous
    ldweights on the PE queue with no intervening PE-state clobber.

    Runs after TileContext exit (schedule + legalize done, semaphores
    assigned) and before nc.compile(). Only ldweights with no semaphore
    ops of their own (sync_info None) are candidates; dangling dependency
    references are remapped onto the kept ldweights.
    """
    removed = {}
    for b in nc.main_func.blocks:
        insts = list(b.instructions)
        keep = []
        prev_key = None
        prev_name = None
        for i in insts:
            tn = type(i).__name__
            if tn == "InstLdweights":
                key = str(i.ins[0])
                if (
                    prev_key is not None
                    and key == prev_key
                    and i.sync_info is None
                ):
                    removed[i.name] = prev_name
                    continue
                prev_key, prev_name = key, i.name
            elif tn == "InstMatmult":
                if getattr(i, "is_transpose", False):
                    prev_key = prev_name = None  # transpose loads the PE
            # other engines' instructions don't touch the PE weight array
            keep.append(i)
        if len(keep) != len(insts):
            b.instructions = keep
    if not removed:
        return 0
    for b in nc.main_func.blocks:
        for i in b.instructions:
            try:
                desc = list(i.descendants)
            except Exception:
                continue
            hit = {n: removed[n] for n in desc if n in removed}
            if hit:
                i.remap_dependency_names(hit)
    for name in removed:
        nc.inst_map.pop(name, None)
    return len(removed)


def _chunks(r_pad):
    out = []
    r = 0
    while r < r_pad:
        out.append((r, min(r + CHUNK, r_pad)))
        r += CHUNK
    return out


def _waves(chunks):
    """Pair consecutive chunks: [[0,1],[2,3],[4]] style."""
    out = []
    i = 0
    while i < len(chunks):
        out.append(list(range(i, min(i + 2, len(chunks)))))
        i += 2
    return out


def build_nc(rt: int, x_dt=None, w_dt=None):
    """Build + compile the per-core Bass program for r_pad = rt*128 rows."""
    x_dt = x_dt or X_DT
    w_dt = w_dt or W_DT
    r_pad = rt * 128
    chunks = _chunks(r_pad)
    waves = _waves(chunks)
    cls = _NoInitBarrierBacc if SKIP_INIT_BARRIER else bacc.Bacc
    nc = cls(
        "TRN2",
        target_bir_lowering=False,
        debug=False,
        enable_asserts=False,
        num_devices=N_CORES,
    )
    f32 = mybir.dt.float32
    xT = nc.dram_tensor("xT", [128, KT * r_pad], x_dt, kind="ExternalInput").ap()
    wT = nc.dram_tensor("wT", [128, KT * D_OUT], w_dt, kind="ExternalInput").ap()
    yT = nc.dram_tensor("yT", [OB, 128, r_pad], Y_DT, kind="ExternalOutput").ap()

    with tile.TileContext(nc) as tc, ExitStack() as ctx:
        w_pool = ctx.enter_context(tc.tile_pool(name="w", bufs=1))
        x_pool = ctx.enter_context(tc.tile_pool(name="x", bufs=1))
        y_pool = ctx.enter_context(tc.tile_pool(name="y", bufs=1))
        p_pool = ctx.enter_context(tc.tile_pool(name="p", bufs=8, space="PSUM"))

        w_sb = w_pool.tile([128, KT * D_OUT], w_dt, tag="w", name="w_sb")
        x_sb = x_pool.tile([128, KT * r_pad], x_dt, tag="x", name="x_sb")
        y_sb = y_pool.tile([128, OB * r_pad], Y_DT, tag="y", name="y_sb")

        # --- input DMAs -------------------------------------------------
        # W on the scalar queue: kt 0-1 first (gates the first matmuls).
        for a, b_ in ((0, 2), (2, 4), (4, KT)):
            nc.scalar.dma_start(
                w_sb[:, a * D_OUT : b_ * D_OUT], wT[:, a * D_OUT : b_ * D_OUT]
            )

        # x arrives kt-major per wave row-span (first-touch order), on the
        # sync and gpsimd queues. The last wave's rows ride with wave 2's.
        def xdma(eng, kt, r0, r1):
            a = kt * r_pad + r0
            b_ = kt * r_pad + r1
            eng.dma_start(x_sb[:, a:b_], xT[:, a:b_])

        spans = []
        if len(waves) >= 2:
            w1_end = chunks[waves[0][-1]][1]
            spans = [(0, w1_end), (w1_end, r_pad)]
        else:
            spans = [(0, r_pad)]
        engs = [nc.sync, nc.gpsimd]
        for r0, r1 in spans:
            for kt in range(KT):
                xdma(engs[kt % 2], kt, r0, r1)

        # --- PE warmup --------------------------------------------------
        if WARMUP_MMS:
            warm_pool = ctx.enter_context(tc.tile_pool(name="warm", bufs=1))
            warm_sb = warm_pool.tile([128, D_OUT], x_dt, tag="warm", name="warm_sb")
            nc.vector.memset(warm_sb[:], 0.0)
            warm_ps = p_pool.tile([128, CHUNK], f32, tag="ps", name="warm_ps")
            for _ in range(WARMUP_MMS):
                nc.tensor.matmul(
                    warm_ps[:], warm_sb[:, :128], warm_sb[:], start=True, stop=True
                )

        # --- waves ------------------------------------------------------
        n_cast = 0
        for wv, cset in enumerate(waves):
            psums = {}
            for ob in range(OB):
                for c in cset:
                    psums[(ob, c)] = p_pool.tile(
                        [128, CHUNK], f32, tag="ps", name=f"ps{wv}_{ob}_{c}"
                    )
            for kt in range(KT):
                for ob in range(OB):
                    lhsT = w_sb[
                        :, kt * D_OUT + ob * 128 : kt * D_OUT + (ob + 1) * 128
                    ]
                    for c in cset:
                        r0, r1 = chunks[c]
                        nc.tensor.matmul(
                            psums[(ob, c)][:, : r1 - r0],
                            lhsT,
                            x_sb[:, kt * r_pad + r0 : kt * r_pad + r1],
                            start=(kt == 0),
                            stop=(kt == KT - 1),
                        )
            # evacuate PSUM (chasing the kt=7 matmuls), then ship the wave
            for ob in range(OB):
                for c in cset:
                    r0, r1 = chunks[c]
                    dst = y_sb[:, ob * r_pad + r0 : ob * r_pad + r1]
                    src = psums[(ob, c)][:, : r1 - r0]
                    # alternate DVE / Activation so the evacuation chases the
                    # kt=7 matmuls and frees banks for the next wave in time
                    # (gpsimd cannot access PSUM on TRN2)
                    if n_cast % 2 == 0:
                        nc.vector.tensor_copy(dst, src)
                    else:
                        nc.scalar.copy(dst, src)
                    n_cast += 1
            r0 = chunks[cset[0]][0]
            r1 = chunks[cset[-1]][1]
            last_wave = wv == len(waves) - 1
            for ob in range(OB):
                eng = nc.sync if last_wave else nc.scalar
                eng.dma_start(
                    yT[ob][:, r0:r1],
                    y_sb[:, ob * r_pad + r0 : ob * r_pad + r1],
                )

    if STRIP_LDW:
        _strip_redundant_ldweights(nc)
    nc.compile()
    return nc


def make_in_maps(x, index, W, x_dt=None, w_dt=None):
    """Group rows by expert, pack per-core k-major transposed tiles.

    Returns (in_maps, rows_per_expert, rt) where rows_per_expert[e] is the
    original row indices handled by core e and rt*128 is the padded row
    count per core.
    """
    x_np = mybir.dt.np(x_dt or X_DT)
    w_np = mybir.dt.np(w_dt or W_DT)
    x = np.ascontiguousarray(x, dtype=np.float32)
    W = np.ascontiguousarray(W, dtype=np.float32)
    rows_per_expert = [np.nonzero(index == e)[0] for e in range(N_CORES)]
    max_rows = max(len(r) for r in rows_per_expert)
    rt = max((max_rows + 127) // 128, 1)
    r_pad = rt * 128

    in_maps = []
    for e in range(N_CORES):
        rows = rows_per_expert[e]
        xp = np.zeros((r_pad, D_IN), np.float32)
        xp[: len(rows)] = x[rows]
        # [R, D_IN] -> [128k, KT, R] so a partition line (fixed k%128) is
        # KT*R elements contiguous, kt-major.
        xT = np.ascontiguousarray(
            xp.reshape(r_pad, KT, 128).transpose(2, 1, 0).reshape(128, KT * r_pad),
            dtype=x_np,
        )
        # W[e] [D_OUT, D_IN] -> [128k, KT, D_OUT] -> [128, KT*D_OUT]
        wT = np.ascontiguousarray(
            W[e].T.reshape(KT, 128, D_OUT).transpose(1, 0, 2).reshape(128, -1),
            dtype=w_np,
        )
        in_maps.append({"xT": xT, "wT": wT})
    return in_maps, rows_per_expert, rt


def assemble_output(results, rows_per_expert, n_rows, index=None, b=None):
    y = np.zeros((n_rows, D_OUT), np.float32)
    for e, rows in enumerate(rows_per_expert):
        yT = results[e]["yT"].reshape(D_OUT, -1)  # [512, r_pad]
        y[rows] = yT[:, : len(rows)].T.astype(np.float32)
    if b is not None and np.any(b):
        y += np.asarray(b, np.float32)[np.asarray(index)]
    return y


def kernel(x, index, W, b):
    x = np.asarray(x)
    index = np.asarray(index, np.int32)
    W = np.asarray(W)
    b = np.asarray(b)
    in_maps, rows_per_expert, rt = make_in_maps(x, index, W)
    nc = build_nc(rt)
    res = run_bass_kernel_spmd(nc, in_maps, core_ids=list(range(N_CORES)))
    return assemble_output(res.results, rows_per_expert, x.shape[0], index, b)
